# revision 1
# baseline (speedup 1.0000x reference)
"""Trainium2 Bass kernel for nn_DecoderPp (PointNet++-style 3-level KNN decoder).

Data-parallel over 16 graphs: core g owns graphs 2g, 2g+1. Per level:
- PE computes s' = -d^2 via K=5 matmuls (positions, |p|^2, |q|^2 folded in).
- DVE max8 (+match_replace for k=16) finds the k-th threshold value exactly.
- DVE reciprocal gives negative-space weights 1/s'; a fused DVE
  scalar_tensor_tensor applies the threshold mask and multiply.
- ACT Copy with a per-partition scale AP normalizes rows by -1/sum(w) and
  casts the dense weight matrix to bf16 (only Copy/Identity/Square/Tanh run
  on ACT -- one activation table, no reloads).
- Per-128 xbar DMA transposes feed bf16 aggregation matmuls y^T = xe^T W^T,
  then the MLP runs feature-major on PE with tanh/bias fused into ACT.
Built on Bacc (finalize() legalizes multi-semaphore waits via EVSEM; raw
Bass trips walrus's one-sync-wait-per-instruction limit). Pool runs only
custom ucode ops on this toolchain, so it is used just for SWDGE loads.
"""
import sys
from contextlib import ExitStack

if "/opt/trn_rl_repo" not in sys.path:
    sys.path.insert(0, "/opt/trn_rl_repo")

import numpy as np

import concourse.bass as bass
import concourse.mybir as mybir
from concourse.bacc import Bacc
from concourse.tile import TileContext
from concourse.masks import make_identity

dt = mybir.dt
AF = mybir.ActivationFunctionType
ALU = mybir.AluOpType

N_CORES = 8
GRAPHS_PER_CORE = 2
N3G, N2G, N1G, N0G = 64, 256, 1024, 4096  # per-graph sizes per level

NEG_BIG = -1.0e30
MASK_THRESH = -1.0e29

f32 = dt.float32
bf16 = dt.bfloat16


def _ceil_div(a, b):
    return (a + b - 1) // b


def build_module(debug=False):
    nc = Bacc()

    P = {}

    def param(name, shape, out=False):
        P[name] = nc.declare_dram_parameter(name, list(shape), f32, isOutput=out)

    param("x", (GRAPHS_PER_CORE * N3G, 256))
    param("pos", (GRAPHS_PER_CORE * N3G, 3))
    param("xs2", (GRAPHS_PER_CORE * N2G, 128))
    param("ps2", (GRAPHS_PER_CORE * N2G, 3))
    param("xs1", (GRAPHS_PER_CORE * N1G, 64))
    param("ps1", (GRAPHS_PER_CORE * N1G, 3))
    param("xs0", (GRAPHS_PER_CORE * N0G, 3))
    param("ps0", (GRAPHS_PER_CORE * N0G, 3))
    for nm, shp in [
        ("W3a", (128, 384)), ("b3a", (128,)),
        ("W3b", (128, 128)), ("b3b", (128,)),
        ("W2a", (64, 192)), ("b2a", (64,)),
        ("W2b", (64, 64)), ("b2b", (64,)),
        ("W1a", (64, 67)), ("b1a", (64,)),
        ("W1b", (64, 64)), ("b1b", (64,)),
        ("W1c", (3, 64)), ("b1c", (3,)),
    ]:
        param(nm, shp)
    param("out", (GRAPHS_PER_CORE * N0G, 3), out=True)
    if debug:
        param("dbg_s3", (128, 64), out=True)
        param("dbg_zap3", (128, 64), out=True)
        param("dbg_v16", (128, 8), out=True)
        param("dbg_sw", (128, 1), out=True)
        param("dbg_w3", (128, 64), out=True)
        param("dbg_W3", (128, 64), out=True)
        param("dbg_qT3", (5, 256), out=True)
        param("dbg_pT3", (5, 64), out=True)
        param("dbg_y3", (128, 128), out=True)
        param("dbg_h3T", (128, 256), out=True)
        param("dbg_h2T", (64, 1024), out=True)
        param("dbg_s1", (128, 1024), out=True)
        param("dbg_zap1", (128, 1024), out=True)
        param("dbg_v16b", (128, 16), out=True)
        param("dbg_sw1", (128, 1), out=True)
        param("dbg_W1", (128, 1024), out=True)
        param("dbg_y1", (64, 128), out=True)
        param("dbg_skc1", (3, 128), out=True)

    with TileContext(nc) as tc, ExitStack() as ctx:
        consts = ctx.enter_context(tc.tile_pool(name="consts", bufs=1))
        wpool = ctx.enter_context(tc.tile_pool(name="weights", bufs=1))
        gpool = ctx.enter_context(tc.tile_pool(name="graph", bufs=2))
        tpool = ctx.enter_context(tc.tile_pool(name="tiles", bufs=5))
        npool = ctx.enter_context(tc.tile_pool(name="narrow", bufs=8))
        pspool = ctx.enter_context(tc.tile_pool(name="ps_s", bufs=2, space="PSUM"))
        psy = ctx.enter_context(tc.tile_pool(name="ps_y", bufs=2, space="PSUM"))
        psm = ctx.enter_context(tc.tile_pool(name="ps_mlp", bufs=1, space="PSUM"))
        pst = ctx.enter_context(tc.tile_pool(name="ps_tp", bufs=1, space="PSUM"))

        ident0 = consts.tile([128, 128], f32)
        make_identity(nc, ident0)
        # ACT-written copy: PE transposes read this so their input waits
        # collapse onto the Activation semaphore (walrus LDW 1-wait limit)
        ident = consts.tile([128, 128], f32)
        nc.scalar.activation(ident[:, :], ident0[:, :], AF.Copy)

        # ---- weight prep: transposed chunks + f32 bias columns.
        # DMA'd weights are staged through an ACT copy so the transpose
        # matmuls wait on a single engine (walrus LDW sync-wait limit). ----
        def prep_linear(wname, bname, O, I, splits, wdtype=bf16):
            w_sb = wpool.tile([O, I], f32, tag=f"{wname}_raw")
            nc.gpsimd.dma_start(w_sb[:, :], P[wname].ap())
            chunks = []
            c0 = 0
            for j, cw in enumerate(splits):
                c1 = c0 + cw
                ps_t = psm.tile([128, 128], f32, tag="mlp")
                nc.tensor.transpose(ps_t[:cw, :O], w_sb[:, c0:c1],
                                    ident[:O, :O])
                wt = wpool.tile([cw, O], wdtype, tag=f"{wname}T{j}")
                nc.scalar.activation(wt[:, :], ps_t[:cw, :O], AF.Copy)
                chunks.append((wt, cw))
                c0 = c1
            bcol = wpool.tile([O, 1], f32, tag=f"{bname}col")
            nc.gpsimd.dma_start(bcol[:, :], P[bname].ap())
            return chunks, bcol

        W3aT, b3a = prep_linear("W3a", "b3a", 128, 384, [128, 128, 128])
        W3bT, b3b = prep_linear("W3b", "b3b", 128, 128, [128])
        W2aT, b2a = prep_linear("W2a", "b2a", 64, 192, [128, 64])
        W2bT, b2b = prep_linear("W2b", "b2b", 64, 64, [64])
        W1aT, b1a = prep_linear("W1a", "b1a", 64, 67, [64, 3])
        W1bT, b1b = prep_linear("W1b", "b1b", 64, 64, [64], wdtype=f32)
        W1cT, b1c = prep_linear("W1c", "b1c", 3, 64, [64], wdtype=f32)

        def load_nat_batch(dram, base, n, d, tag):
            """One DMA: dram rows [base:base+n, :d] -> [128, (n//128)*d]."""
            a = n // 128
            t = gpool.tile([128, a * d], f32, tag=tag)
            src_ap = dram.ap()[base : base + n, :].rearrange(
                "(a p) d -> p a d", p=128)
            nc.gpsimd.dma_start(t[:, :], src_ap)
            return t

        def pos5_chunk(pn, rows, scale3, sq_col, one_col, sq_scale, dst, dc0):
            """Build [rows,5] = columns of scaled pos, -|p|^2 (at sq_col,
            scaled), and 1 (at one_col) from preloaded natural pos [rows,3];
            transpose on PE and copy into dst[:, dc0:dc0+rows]."""
            p5 = tpool.tile([128, 5], f32, tag="pos5")
            nc.scalar.activation(p5[:rows, 0:3], pn[:rows, :], AF.Copy,
                                 scale=scale3)
            sqs = tpool.tile([128, 3], f32, tag="possq")
            ppc = npool.tile([128, 1], f32, tag="ppc")
            nc.scalar.activation(sqs[:rows, :], pn[:rows, :], AF.Square,
                                 accum_out=ppc[:rows, :])
            nc.scalar.activation(p5[:rows, sq_col : sq_col + 1], ppc[:rows, :],
                                 AF.Copy, scale=sq_scale)
            # ones column via ACT (keep all p5 writers on one engine)
            nc.scalar.activation(p5[:rows, one_col : one_col + 1],
                                 ppc[:rows, :], AF.Copy, scale=0.0, bias=1.0)
            t_ps = pst.tile([128, 128], f32, tag="tpos")
            nc.tensor.transpose(t_ps[:5, :rows], p5[:rows, 0:5],
                                ident[:rows, :rows])
            nc.scalar.activation(dst[:, dc0 : dc0 + rows], t_ps[:5, :rows],
                                 AF.Copy)

        # ---------------- one interpolation+MLP level ----------------
        def prop_level(g, lvl, ns, nt, k, Cs, xe_chunks, p_dram, q_dram,
                       skip_dram, Ck, mlp, out_tile):
            """mlp: list of (chunks, bcol, tanh?, O, out_dtype)."""
            # p-side [5, ns]: rows = [p^T; -|p|^2; 1] assembled per 128-chunk
            pT = gpool.tile([5, ns], f32, tag=f"pT{lvl}")
            if ns >= 128:
                pnb = load_nat_batch(p_dram, g * ns, ns, 3, f"pnb{lvl}")
            else:
                pnb = gpool.tile([128, 3], f32, tag=f"pnb{lvl}")
                nc.gpsimd.dma_start(pnb[:ns, :],
                                  p_dram.ap()[g * ns : (g + 1) * ns, :])
            qnb = load_nat_batch(q_dram, g * nt, nt, 3, f"qnb{lvl}")
            sknb = None
            if Ck <= 4:
                sknb = load_nat_batch(skip_dram, g * nt, nt, Ck, f"sknb{lvl}")
            for ci in range(_ceil_div(ns, 128)):
                rows = min(128, ns - ci * 128)
                pos5_chunk(pnb[:, 3 * ci : 3 * ci + 3], rows, 1.0, 3, 4, -1.0,
                           pT, ci * 128)

            ns_pad = max(128, ns)
            n_sch = _ceil_div(ns, 128)
            nfc = _ceil_div(Cs, 128)

            for ti in range(nt // 128):
                t0 = ti * 128
                # q lhsT [5,128]: rows = [2q^T; 1; -|q|^2]
                qlhs = tpool.tile([5, 128], f32, tag="qlhs")
                pos5_chunk(qnb[:, 3 * ti : 3 * ti + 3], 128, 2.0, 4, 3, -1.0,
                           qlhs, 0)
                # s' = -d2 : [128, ns] PSUM (K=5 matmul)
                s_ps = pspool.tile([128, 1024], f32, tag="s")
                for h0 in range(0, ns, 512):
                    h1 = min(ns, h0 + 512)
                    nc.tensor.matmul(s_ps[:, h0:h1], qlhs[:, :],
                                     pT[:, h0:h1], start=True, stop=True)
                sv = s_ps[:, :ns]

                # --- selection (values only) ---
                v16 = npool.tile([128, 16], f32, tag="v16")
                nc.vector.max(v16[:, 0:8], sv)
                if k == 16:
                    zap = tpool.tile([128, ns_pad], f32, tag="zap")
                    nc.vector.match_replace(zap[:, :ns], v16[:, 0:8], sv,
                                            NEG_BIG)
                    nc.vector.max(v16[:, 8:16], zap[:, :ns])

                # --- dense weights via reciprocal in negative space ---
                # wneg = 1/s' = -1/d2 < 0; selected iff wneg <= 1/v_k
                wneg = tpool.tile([128, ns_pad], f32, tag="wneg")
                nc.vector.reciprocal(wneg[:, :ns], sv)
                taur = npool.tile([128, 1], f32, tag="taur")
                nc.vector.reciprocal(taur[:, :], v16[:, k - 1 : k])
                Wraw = tpool.tile([128, ns_pad], f32, tag="Wraw")
                swneg = npool.tile([128, 1], f32, tag="swneg")
                nc.vector.scalar_tensor_tensor(
                    Wraw[:, :ns], wneg[:, :ns], taur[:, :], wneg[:, :ns],
                    op0=ALU.is_le, op1=ALU.mult, accum_out=swneg[:, :])
                swrec = npool.tile([128, 1], f32, tag="swrec")
                nc.vector.reciprocal(swrec[:, :], swneg[:, :])
                W = tpool.tile([128, ns_pad], bf16, tag="W")
                if ns < ns_pad:
                    nc.vector.memset(W[:, ns:], 0.0)
                # W = Wraw * (1/sum(Wraw)) -- negatives cancel
                nc.scalar.activation(W[:, :ns], Wraw[:, :ns], AF.Copy,
                                     scale=swrec[:, :])

                if debug and g == 0 and lvl == 1 and ti == 0:
                    stg1 = tpool.tile([128, 1024], f32, tag="dbgstg1")
                    nc.scalar.activation(stg1[:, :], s_ps[:, :], AF.Copy)
                    nc.sync.dma_start(P["dbg_s1"].ap(), stg1[:, :])
                    nc.sync.dma_start(P["dbg_v16b"].ap(), v16[:, :])
                    nc.sync.dma_start(P["dbg_sw1"].ap(), sw[:, :])
                    nc.gpsimd.dma_start(P["dbg_W1"].ap(), W[:, :])
                if debug and g == 0 and lvl == 3 and ti == 0:
                    stg = tpool.tile([128, 64], f32, tag="dbgstg")
                    nc.scalar.activation(stg[:, :], s_ps[:, :64], AF.Copy)
                    nc.sync.dma_start(P["dbg_s3"].ap(), stg[:, :])
                    nc.sync.dma_start(P["dbg_v16"].ap(), v16[:, 0:8])
                    nc.sync.dma_start(P["dbg_sw"].ap(), sw[:, :])
                    nc.sync.dma_start(P["dbg_w3"].ap(), Wraw[:, :64])
                    nc.gpsimd.dma_start(P["dbg_W3"].ap(), W[:, :64])

                # --- transpose W chunks; aggregate y^T = xe^T @ W^T ---
                WT = []
                for j in range(ns_pad // 128):
                    wt = tpool.tile([128, 128], bf16, tag=f"WT{j}")
                    nc.sync.dma_start_transpose(
                        wt[:, :], W[:, j * 128 : (j + 1) * 128])
                    WT.append(wt)

                y_ps = []
                for fc in range(nfc):
                    f0, f1 = fc * 128, min(Cs, (fc + 1) * 128)
                    yp = psy.tile([128, 128], f32, tag="y")
                    for j in range(n_sch):
                        kr = min(128, ns - j * 128)
                        nc.tensor.matmul(yp[: f1 - f0, :],
                                         xe_chunks[j][0][:kr, f0:f1],
                                         WT[j][:kr, :],
                                         start=(j == 0), stop=(j == n_sch - 1))
                    y_ps.append((yp, f1 - f0))

                # --- MLP input chunks: y^T (bf16) + skip^T ---
                in_chunks = []
                for fc, (yp, fw) in enumerate(y_ps):
                    hc = tpool.tile([128, 128], bf16, tag=f"hc{fc}")
                    nc.scalar.activation(hc[:fw, :], yp[:fw, :], AF.Copy)
                    in_chunks.append((hc, fw))
                if Ck <= 4:
                    sk_ps = pst.tile([128, 128], f32, tag="tpos")
                    nc.tensor.transpose(sk_ps[:Ck, :],
                                        sknb[:, Ck * ti : Ck * ti + Ck],
                                        ident[:, :])
                    skc = tpool.tile([Ck, 128], bf16, tag="skc")
                    nc.scalar.activation(skc[:, :], sk_ps[:Ck, :], AF.Copy)
                    in_chunks.append((skc, Ck))
                else:
                    sk_nat = tpool.tile([128, 128], bf16, tag="sknat")
                    base = g * nt
                    nc.gpsimd.dma_start(
                        sk_nat[:, :Ck],
                        skip_dram.ap()[base + t0 : base + t0 + 128, :])
                    if Ck < 128:
                        nc.vector.memset(sk_nat[:, Ck:], 0.0)
                    skc = tpool.tile([128, 128], bf16, tag="skc")
                    nc.sync.dma_start_transpose(skc[:, :], sk_nat[:, :])
                    in_chunks.append((skc, Ck))

                if debug and g == 0 and lvl == 3 and ti == 0:
                    nc.gpsimd.dma_start(P["dbg_y3"].ap(), in_chunks[0][0][:, :])
                if debug and g == 0 and lvl == 1 and ti == 0:
                    nc.gpsimd.dma_start(P["dbg_y1"].ap(), in_chunks[0][0][:64, :])
                    nc.gpsimd.dma_start(P["dbg_skc1"].ap(), in_chunks[1][0][:, :])

                # --- MLP (feature-major) ---
                cur = in_chunks
                for li, (chunks, bcol, tanh, O, odt) in enumerate(mlp):
                    mp = psm.tile([128, 128], f32, tag="mlp")
                    nkc = len(cur)
                    for j, (ct, kr) in enumerate(cur):
                        wt, cw = chunks[j]
                        assert cw == kr, f"l{lvl} mlp{li} c{j}: {cw} != {kr}"
                        nc.tensor.matmul(mp[:O, :], wt[:, :O], ct[:kr, :],
                                         start=(j == 0), stop=(j == nkc - 1))
                    if li == len(mlp) - 1:
                        nc.scalar.activation(out_tile[:O, t0 : t0 + 128],
                                             mp[:O, :], AF.Identity,
                                             bias=bcol[:, :])
                    else:
                        ho = tpool.tile([128, 128], odt, tag=f"ho{li}")
                        nc.scalar.activation(ho[:O, :], mp[:O, :],
                                             AF.Tanh if tanh else AF.Identity,
                                             bias=bcol[:, :])
                        cur = [(ho, O)]

        # ---------------- per-graph pipeline ----------------
        for g in range(GRAPHS_PER_CORE):
            # level 3: x[64,256] -> h3 [256,128]
            xe3f = gpool.tile([64, 256], f32, tag="xe3f")
            nc.gpsimd.dma_start(xe3f[:, :], P["x"].ap()[g * 64 : (g + 1) * 64, :])
            xe3 = gpool.tile([64, 256], bf16, tag="xe3")
            nc.scalar.activation(xe3[:, :], xe3f[:, :], AF.Copy)
            h3T = gpool.tile([128, 256], bf16, tag="h3T")
            prop_level(g, 3, N3G, N2G, 4, 256, [(xe3, 64)], P["pos"], P["ps2"],
                       P["xs2"], 128,
                       [(W3aT, b3a, True, 128, bf16),
                        (W3bT, b3b, False, 128, bf16)], h3T)
            if debug and g == 0:
                nc.gpsimd.dma_start(P["dbg_h3T"].ap(), h3T[:, :])
            h3nat = []
            for j in range(2):
                hn = gpool.tile([128, 128], bf16, tag=f"h3n{j}")
                nc.sync.dma_start_transpose(hn[:, :],
                                            h3T[:, j * 128 : (j + 1) * 128])
                h3nat.append((hn, 128))

            # level 2: h3 [256,128] -> h2 [1024,64]
            h2T = gpool.tile([64, 1024], bf16, tag="h2T")
            prop_level(g, 2, N2G, N1G, 8, 128, h3nat, P["ps2"], P["ps1"],
                       P["xs1"], 64,
                       [(W2aT, b2a, True, 64, bf16),
                        (W2bT, b2b, False, 64, bf16)], h2T)
            if debug and g == 0:
                nc.gpsimd.dma_start(P["dbg_h2T"].ap(), h2T[:, :])
            h2nat = []
            for j in range(8):
                hn = gpool.tile([128, 64], bf16, tag=f"h2n{j}")
                nc.sync.dma_start_transpose(hn[:, :],
                                            h2T[:, j * 128 : (j + 1) * 128])
                h2nat.append((hn, 128))

            # level 1: h2 [1024,64] -> out [4096,3]
            outT = gpool.tile([3, 4096], f32, tag="outT")
            prop_level(g, 1, N1G, N0G, 16, 64, h2nat, P["ps1"], P["ps0"],
                       P["xs0"], 3,
                       [(W1aT, b1a, True, 64, f32),
                        (W1bT, b1b, True, 64, f32),
                        (W1cT, b1c, False, 3, f32)], outT)
            base = g * N0G
            for i in range(3):
                nc.sync.dma_start(P["out"].ap()[base : base + N0G, i],
                                  outT[i : i + 1, :])

    return nc, P


_NC = None


def _get_nc():
    global _NC
    if _NC is None:
        nc = build_module()[0]
        nc.finalize()  # Bacc lowering: EVSEM wait legalization + reg alloc
        _NC = nc
    return _NC


def shard_inputs(inputs):
    f = lambda name: np.ascontiguousarray(np.asarray(inputs[name], np.float32))
    arrs = {
        "x": (f("x"), N3G), "pos": (f("pos"), N3G),
        "xs2": (f("x_skip2"), N2G), "ps2": (f("pos_skip2"), N2G),
        "xs1": (f("x_skip1"), N1G), "ps1": (f("pos_skip1"), N1G),
        "xs0": (f("x_skip0"), N0G), "ps0": (f("pos_skip0"), N0G),
    }
    weights = {k: f(k) for k in ["W3a", "b3a", "W3b", "b3b", "W2a", "b2a",
                                 "W2b", "b2b", "W1a", "b1a", "W1b", "b1b",
                                 "W1c", "b1c"]}
    in_maps = []
    for c in range(N_CORES):
        m = dict(weights)
        for nm, (arr, ng) in arrs.items():
            m[nm] = np.ascontiguousarray(
                arr[2 * c * ng : (2 * c + 2) * ng])
        in_maps.append(m)
    return in_maps


def kernel(**inputs):
    nc = _get_nc()
    in_maps = shard_inputs(inputs)
    from concourse.bass_utils import run_bass_kernel_spmd

    res = run_bass_kernel_spmd(nc, in_maps, list(range(N_CORES)))
    return np.concatenate([np.asarray(r["out"], np.float32)
                           for r in res.results], axis=0)


if __name__ == "__main__":
    nc, _ = build_module()
    print("build ok")



# revision 29
# speedup vs baseline: 1.6658x; 1.6658x over previous
"""Trainium2 Bass kernel for nn_DecoderPp (PointNet++-style 3-level KNN decoder).

Data-parallel over 16 graphs: core g owns graphs 2g, 2g+1. Per level:
- PE computes s' = -d^2 via K=5 f32 matmuls (positions, |p|^2, |q|^2 folded).
- ACT computes dense weights wpos = exp(-ln(-s')) = 1/d^2 (log/exp live in one
  activation table; tanh lives in another, so interp and MLP are phase-ordered
  to bound table reloads).
- Selection runs on wpos (positive, SBUF): nearest = largest. k=4/8 need one
  max8; k=16 uses 8 block max8s + exact top8-of-64 + match_replace(0) + max8
  (zap value 0 is safe in positive space). ~0.05% of targets may pick a
  slightly-off 16th neighbour (a block holding >8 of the true top-16).
- One DVE scalar_tensor_tensor masks: W = (wpos >= wtau) * wpos, bf16.
- Aggregation is target-major: y[128, C+1] = sum_j WT_j^T @ xe_aug_j where
  xe_aug carries a trailing ones column so y[:, C] = sum(w) for free; W^T
  chunks come from xbar DMA transposes.
- ACT normalizes y rows by 1/sumw into haug bf16; skip features are DMA-cast
  straight into haug's tail columns; PE transposes per feature-chunk build the
  feature-major MLP input hT, drained from PSUM by casting SWDGE (Pool).
- MLPs run feature-major over 512-column blocks with tanh/bias fused on ACT.
Built on Bacc (finalize() legalizes multi-semaphore waits via EVSEM).
"""
import sys
from contextlib import ExitStack

if "/opt/trn_rl_repo" not in sys.path:
    sys.path.insert(0, "/opt/trn_rl_repo")

import numpy as np

import concourse.bass as bass
import concourse.mybir as mybir
from concourse.bacc import Bacc
from concourse.tile import TileContext
from concourse.masks import make_identity

dt = mybir.dt
AF = mybir.ActivationFunctionType
ALU = mybir.AluOpType

N_CORES = 8
GRAPHS_PER_CORE = 2
N3G, N2G, N1G, N0G = 64, 256, 1024, 4096  # per-graph sizes per level

f32 = dt.float32
bf16 = dt.bfloat16


def build_module(exact_sel=False):
    nc = Bacc()

    P = {}

    def param(name, shape, out=False):
        P[name] = nc.declare_dram_parameter(name, list(shape), f32, isOutput=out)

    param("x", (GRAPHS_PER_CORE * N3G, 256))
    param("pos", (GRAPHS_PER_CORE * N3G, 3))
    param("xs2", (GRAPHS_PER_CORE * N2G, 128))
    param("ps2", (GRAPHS_PER_CORE * N2G, 3))
    param("xs1", (GRAPHS_PER_CORE * N1G, 64))
    param("ps1", (GRAPHS_PER_CORE * N1G, 3))
    param("xs0", (GRAPHS_PER_CORE * N0G, 3))
    param("ps0", (GRAPHS_PER_CORE * N0G, 3))
    for nm, shp in [
        ("W3a", (128, 384)), ("b3a", (128,)),
        ("W3b", (128, 128)), ("b3b", (128,)),
        ("W2a", (64, 192)), ("b2a", (64,)),
        ("W2b", (64, 64)), ("b2b", (64,)),
        ("W1a", (64, 67)), ("b1a", (64,)),
        ("W1b", (64, 64)), ("b1b", (64,)),
        ("W1c", (3, 64)), ("b1c", (3,)),
    ]:
        param(nm, shp)
    param("out", (GRAPHS_PER_CORE * N0G, 3), out=True)

    with TileContext(nc) as tc, ExitStack() as ctx:
        consts = ctx.enter_context(tc.tile_pool(name="consts", bufs=1))
        wpool = ctx.enter_context(tc.tile_pool(name="weights", bufs=1))
        gpool = ctx.enter_context(tc.tile_pool(name="graph", bufs=2))
        tpool = ctx.enter_context(tc.tile_pool(name="tiles", bufs=3))
        npool = ctx.enter_context(tc.tile_pool(name="narrow", bufs=8))
        psS = ctx.enter_context(tc.tile_pool(name="ps_s", bufs=2, space="PSUM"))
        psY = ctx.enter_context(tc.tile_pool(name="ps_y", bufs=1, space="PSUM"))
        psT = ctx.enter_context(tc.tile_pool(name="ps_tp", bufs=1, space="PSUM"))
        psM = ctx.enter_context(tc.tile_pool(name="ps_mlp", bufs=1, space="PSUM"))

        ident = consts.tile([128, 128], f32)
        make_identity(nc, ident)
        identb = consts.tile([128, 128], bf16)
        nc.scalar.activation(identb[:, :], ident[:, :], AF.Copy)

        # ---- weight prep: bf16 natural via casting DMA, then xbar
        # transposes into [cw, O] lhsT chunks; f32 bias columns. ----
        def prep_linear(wname, bname, O, I, splits):
            # xbar transpose needs partitions %16 == 0 and free %128 == 0
            Op = (O + 15) // 16 * 16
            Ip = (I + 127) // 128 * 128
            wbf = wpool.tile([Op, Ip], bf16, name=f"{wname}bf",
                             tag=f"{wname}bf")
            if Op != O or Ip != I:
                nc.vector.memset(wbf[:, :], 0.0)
            nc.gpsimd.dma_start(wbf[:O, :I], P[wname].ap())
            chunks = []
            c0 = 0
            for j, cw in enumerate(splits):
                wt = wpool.tile([128, Op], bf16, name=f"{wname}T{j}",
                                tag=f"{wname}T{j}")
                nc.sync.dma_start_transpose(
                    wt[:, :], wbf[:, j * 128 : (j + 1) * 128])
                chunks.append((wt, cw))
                c0 += cw
            bcol = wpool.tile([O, 1], f32, name=f"{bname}c", tag=f"{bname}c")
            nc.sync.dma_start(bcol[:, :], P[bname].ap())
            return chunks, bcol


        # ---- [5, n] position-feature rows from natural [n, 3] ----
        # q-side rows: [2q; -|q|^2; 1], p-side rows: [p; 1; -|p|^2]
        _p5n = [0]
        KD = 19  # hi/lo bf16 rows: 4x3 cross, 3+3 split norms, 1 eps floor

        def build_pos18(dram, base, n, qside, tag, use_dve=False):
            """[18, n] bf16 position rows from natural [n, 3] f32.
            Pairing (q-row, p-row): (-2qh,ph)x3 (-2ql,ph)x3 (-2qh,pl)x3
            (-2ql,pl)x3 (q2a,1)(q2b,1)(q2c,1) (1,p2a)(1,p2b)(1,p2c), so
            q18^T . p18 = |q-p|^2 with ~1e-5 abs error (products of bf16
            pairs are exact; only the residual roundings remain)."""
            _p5n[0] += 1
            tag = tag + ("q" if qside else "p")
            uid = f"{tag}_{_p5n[0]}"
            rows = min(n, 128)
            a = max(1, n // 128)
            pn = gpool.tile([rows, a * 3], f32, name=f"pn{uid}", tag=f"pn{tag}")
            if n >= 128:
                nc.sync.dma_start(
                    pn[:, :],
                    dram.ap()[base : base + n, :].rearrange(
                        "(a p) d -> p a d", p=128))
            else:
                nc.sync.dma_start(pn[:, :], dram.ap()[base : base + n, :])
            ph = gpool.tile([rows, a * 3], bf16, name=f"ph{uid}", tag=f"ph{tag}")
            if use_dve:
                nc.vector.tensor_copy(ph[:, :], pn[:, :])
            else:
                nc.scalar.activation(ph[:, :], pn[:, :], AF.Copy)
            pl = gpool.tile([rows, a * 3], bf16, name=f"pl{uid}", tag=f"pl{tag}")
            nc.vector.tensor_tensor(pl[:, :], pn[:, :], ph[:, :],
                                    op=ALU.subtract)
            sq = gpool.tile([rows, a * 3], f32, name=f"sq{uid}", tag=f"sq{tag}")
            if use_dve:
                nc.vector.tensor_tensor(sq[:, :], pn[:, :], pn[:, :],
                                        op=ALU.mult)
            else:
                nc.scalar.activation(sq[:, :], pn[:, :], AF.Square)
            sq3 = sq[:, :].rearrange("p (a d) -> p a d", d=3)
            qq = gpool.tile([rows, a], f32, name=f"qq{uid}", tag=f"qq{tag}")
            nc.vector.tensor_tensor(qq[:, :], sq3[:, :, 0], sq3[:, :, 1],
                                    op=ALU.add)
            nc.vector.tensor_tensor(qq[:, :], qq[:, :], sq3[:, :, 2],
                                    op=ALU.add)
            # 3-way split of the squared norm
            n2a = gpool.tile([rows, a], bf16, name=f"n2a{uid}", tag=f"n2a{tag}")
            if use_dve:
                nc.vector.tensor_copy(n2a[:, :], qq[:, :])
            else:
                nc.scalar.activation(n2a[:, :], qq[:, :], AF.Copy)
            r1 = gpool.tile([rows, a], f32, name=f"r1{uid}", tag=f"r1{tag}")
            nc.vector.tensor_tensor(r1[:, :], qq[:, :], n2a[:, :],
                                    op=ALU.subtract)
            n2b = gpool.tile([rows, a], bf16, name=f"n2b{uid}", tag=f"n2b{tag}")
            if use_dve:
                nc.vector.tensor_copy(n2b[:, :], r1[:, :])
            else:
                nc.scalar.activation(n2b[:, :], r1[:, :], AF.Copy)
            n2c = gpool.tile([rows, a], bf16, name=f"n2c{uid}", tag=f"n2c{tag}")
            nc.vector.tensor_tensor(n2c[:, :], r1[:, :], n2b[:, :],
                                    op=ALU.subtract)
            p5 = gpool.tile([rows, a * KD], bf16, name=f"p5{uid}",
                            tag=f"p5{tag}")
            p5v = p5[:, :].rearrange("p (a d) -> p a d", d=KD)
            hsc = -2.0 if qside else 1.0
            ph3 = ph[:, :].rearrange("p (a d) -> p a d", d=3)
            pl3 = pl[:, :].rearrange("p (a d) -> p a d", d=3)

            def scat(dst, src, scale):
                if use_dve:
                    if scale == 1.0:
                        nc.vector.tensor_copy(dst, src)
                    else:
                        nc.vector.tensor_scalar(dst, src, scale, None,
                                                op0=ALU.mult)
                else:
                    nc.scalar.activation(dst, src, AF.Copy, scale=scale)

            if qside:
                scat(p5v[:, :, 0:3], ph3[:, :, :], hsc)
                scat(p5v[:, :, 3:6], pl3[:, :, :], hsc)
                scat(p5v[:, :, 6:9], ph3[:, :, :], hsc)
                scat(p5v[:, :, 9:12], pl3[:, :, :], hsc)
            else:
                scat(p5v[:, :, 0:3], ph3[:, :, :], 1.0)
                scat(p5v[:, :, 3:6], ph3[:, :, :], 1.0)
                scat(p5v[:, :, 6:9], pl3[:, :, :], 1.0)
                scat(p5v[:, :, 9:12], pl3[:, :, :], 1.0)
            sq0, one0 = (12, 15) if qside else (15, 12)
            for i, t in enumerate((n2a, n2b, n2c)):
                scat(p5v[:, :, sq0 + i : sq0 + i + 1],
                     t[:, :].rearrange("p (a d) -> p a d", d=1), 1.0)
            nc.vector.memset(p5v[:, :, one0 : one0 + 3], 1.0)
            # row 18: (eps, 1) pair floors s at +eps so ln never sees <= 0
            nc.vector.memset(p5v[:, :, 18:19], 1.0)
            if qside:
                scat(p5v[:, :, 18:19], p5v[:, :, 18:19], 2e-4)
            pT = gpool.tile([KD, n], bf16, name=f"pT{uid}", tag=f"pT{tag}")
            c = 0
            while c < a:
                w = min(2, a - c)
                tp = psT.tile([128, 256], bf16, name=f"tp5{uid}{c}", tag="tp5")
                for j in range(w):
                    nc.tensor.transpose(
                        tp[:KD, j * 128 : j * 128 + rows],
                        p5[:, (c + j) * KD : (c + j) * KD + KD],
                        identb[:rows, :rows])
                wd = 256 if w == 2 else rows
                nc.scalar.activation(pT[:, c * 128 : c * 128 + wd],
                                     tp[:KD, :wd], AF.Copy)
                c += w
            return pT

        # ---------------- interpolation level ----------------
        def interp_level(g, lvl, ns, nt, k, Cs, Ck, xe_chunks, q_dram, pT,
                         skip_dram, hT_chunks, tokZ=None, n_dve=(0, 0)):
            n_sch = max(1, ns // 128)
            Ctot = Cs + Ck
            fsplits = []
            f0 = 0
            while f0 < Ctot:
                fw = min(128, Ctot - f0)
                fsplits.append((f0, fw))
                f0 += fw
            pend = {}
            sub = min(nt, 1024)
            skipbf = gpool.tile([128, (nt // 128) * Ck], bf16,
                                name=f"skb{lvl}{g}", tag=f"skb{lvl}")
            nc.gpsimd.dma_start(
                skipbf[:, :],
                skip_dram.ap()[g * nt : (g + 1) * nt, :].rearrange(
                    "(a p) d -> p a d", p=128))
            q5T_next = build_pos18(q_dram, g * nt, sub, True, f"q{lvl}",
                                   use_dve=True)
            for r0 in range(0, nt, sub):
                q5T = q5T_next
                if r0 + sub < nt:
                    q5T_next = build_pos18(q_dram, g * nt + r0 + sub, sub,
                                           True, f"q{lvl}",
                                           use_dve=(lvl != 1 or g == 0 and
                                                    r0 == 0))
                interp_sub(g, lvl, ns, nt, k, Cs, Ck, Ctot, xe_chunks, q5T,
                           pT, skip_dram, hT_chunks, fsplits, pend, r0, sub,
                           n_sch, skipbf, tokZ, n_dve)

        def interp_sub(g, lvl, ns, nt, k, Cs, Ck, Ctot, xe_chunks, q5T, pT,
                       skip_dram, hT_chunks, fsplits, pend, r0, sub, n_sch,
                       skipbf, tokZ, n_dve):
            for ti in range(r0 // 128, (r0 + sub) // 128):
                t0 = ti * 128
                s = psS.tile([128, 1024], f32, name=f"s{lvl}{g}{ti}", tag="s")
                for h0 in range(0, ns, 512):
                    h1 = min(ns, h0 + 512)
                    nc.tensor.matmul(s[:, h0:h1],
                                     q5T[:, t0 - r0 : t0 - r0 + 128],
                                     pT[:, h0:h1], start=True, stop=True)

                wpos = tpool.tile([128, ns], f32, name=f"wp{lvl}{g}{ti}",
                                  tag=f"wp{lvl}", bufs=2)
                nchunks = nt // 128
                head, tail = n_dve
                if ti < head or ti >= nchunks - tail:
                    # DVE reciprocal path: no table-bound ACT ops, so these
                    # chunks overlap freely with the previous tanh phase
                    nc.vector.reciprocal(wpos[:, :], s[:, :ns])
                else:
                    L = tpool.tile([128, ns], f32, name=f"L{lvl}{g}{ti}",
                                   tag=f"L{lvl}", bufs=2)
                    nc.scalar.activation(
                        L[:, :], s[:, :ns], AF.Ln,
                        bias=tokZ[:, 0:1] if tokZ is not None else 0.0)
                    nc.scalar.activation(wpos[:, :], L[:, :], AF.Exp,
                                         scale=-1.0)

                if k == 16 and not exact_sel:
                    cand = tpool.tile([128, 64], f32, name=f"cd{g}{ti}",
                                      tag="cand")
                    for b in range(8):
                        nc.vector.max(cand[:, 8 * b : 8 * b + 8],
                                      wpos[:, 128 * b : 128 * (b + 1)])
                    v8 = npool.tile([128, 8], f32, name=f"v8{g}{ti}", tag="v8")
                    nc.vector.max(v8[:, :], cand[:, :])
                    zap = tpool.tile([128, 64], f32, name=f"zp{g}{ti}",
                                     tag="zap")
                    nc.vector.match_replace(zap[:, :], v8[:, :], cand[:, :],
                                            0.0)
                    v16 = npool.tile([128, 8], f32, name=f"v16{g}{ti}",
                                     tag="v16")
                    nc.vector.max(v16[:, :], zap[:, :])
                    wtau = v16[:, 7:8]
                elif k == 16:
                    v8 = npool.tile([128, 8], f32, name=f"v8{g}{ti}", tag="v8")
                    nc.vector.max(v8[:, :], wpos[:, :])
                    zap = tpool.tile([128, ns], f32, name=f"zp{g}{ti}",
                                     tag="zap")
                    nc.vector.match_replace(zap[:, :], v8[:, :], wpos[:, :],
                                            0.0)
                    v16 = npool.tile([128, 8], f32, name=f"v16{g}{ti}",
                                     tag="v16")
                    nc.vector.max(v16[:, :], zap[:, :])
                    wtau = v16[:, 7:8]
                else:
                    v8 = npool.tile([128, 8], f32, name=f"v8{g}{ti}", tag="v8")
                    nc.vector.max(v8[:, :], wpos[:, :])
                    wtau = v8[:, k - 1 : k]

                ns_pad = max(128, ns)
                W = tpool.tile([128, ns_pad], bf16, name=f"W{lvl}{g}{ti}",
                               tag=f"W{lvl}")
                nc.vector.scalar_tensor_tensor(W[:, :ns], wpos[:, :], wtau,
                                               wpos[:, :], op0=ALU.is_ge,
                                               op1=ALU.mult)
                if ns < ns_pad:
                    nc.vector.memset(W[:, ns:], 0.0)
                WT = []
                for j in range(n_sch):
                    kr = min(128, ns)
                    wt = tpool.tile([128, 128], bf16, name=f"WT{j}g{g}{ti}",
                                    tag=f"WT{lvl}_{j}")
                    nc.sync.dma_start_transpose(
                        wt[:, :], W[:, j * 128 : (j + 1) * 128])
                    WT.append((wt, kr))

                y = psY.tile([128, 257], f32, name=f"y{lvl}{g}{ti}", tag="y")
                for j, (wt, kr) in enumerate(WT):
                    nc.tensor.matmul(
                        y[:, : Cs + 1], wt[:kr, :],
                        xe_chunks[j][0][:kr, : Cs + 1],
                        start=(j == 0), stop=(j == n_sch - 1))
                sw = npool.tile([128, 1], f32, name=f"sw{g}{ti}", tag="sw")
                nc.vector.reciprocal(sw[:, :], y[:, Cs : Cs + 1])
                haug = tpool.tile([128, Ctot], bf16, name=f"ha{lvl}{g}{ti}",
                                  tag=f"ha{lvl}")
                if lvl == 1:
                    nc.scalar.activation(haug[:, :Cs], y[:, :Cs], AF.Copy,
                                         scale=sw[:, :])
                else:
                    nc.vector.tensor_scalar(haug[:, :Cs], y[:, :Cs],
                                            sw[:, :], None, op0=ALU.mult)
                nc.sync.dma_start(haug[:, Cs:Ctot],
                                  skipbf[:, ti * Ck : (ti + 1) * Ck])

                # transpose haug -> hT; batch 2 chunks per drain when the
                # psum period tile (1 bank) has room
                period = max(1, 512 // (128 * len(fsplits)))
                pw = 128 * period
                if not pend.get("t0s"):
                    pend["tile"] = psT.tile([128, pw * len(fsplits)], bf16,
                                            name=f"tph{lvl}{g}{ti}",
                                            tag="tph")
                    pend["t0s"] = []
                tp = pend["tile"]
                npos = len(pend["t0s"])
                for fi, (f0, fw) in enumerate(fsplits):
                    nc.tensor.transpose(
                        tp[:fw, fi * pw + npos * 128 :
                           fi * pw + npos * 128 + 128],
                        haug[:, f0 : f0 + fw], identb[:, :])
                pend["t0s"].append(t0)
                if len(pend["t0s"]) == period or ti == nt // 128 - 1:
                    nb = len(pend["t0s"])
                    for fi, (f0, fw) in enumerate(fsplits):
                        ht = hT_chunks[fi][0]
                        nc.scalar.activation(
                            ht[:, pend["t0s"][0] : pend["t0s"][0] + nb * 128],
                            tp[:fw, fi * pw : fi * pw + nb * 128], AF.Copy)
                    pend["t0s"] = []

        # ---------------- MLP over feature-major hT ----------------
        def mlp_phase(g, lvl, hT_chunks, nt, layers, blk=512, stream_g=0,
                      tokS=None):
            """layers: (chunks, bcol, O, act, dst). dst None = per-block
            scratch (consumed by the next layer); dst tile = full-width."""
            for b0 in range(0, nt, blk):
                bw = min(blk, nt - b0)
                cur = hT_chunks
                cc0 = b0
                for li, (chunks, bcol, O, act, dst) in enumerate(layers):
                    mp = psM.tile([128, 512], f32, name=f"m{lvl}{g}{b0}{li}",
                                  tag="mlp")
                    nkc = len(cur)
                    for j, (ct, kr) in enumerate(cur):
                        wt, cw = chunks[j]
                        nc.tensor.matmul(mp[:O, :bw], wt[:kr, :O],
                                         ct[:kr, cc0 : cc0 + bw],
                                         start=(j == 0), stop=(j == nkc - 1))
                    if dst is None:
                        ho = tpool.tile([O, blk], bf16,
                                        name=f"h{lvl}{g}{b0}_{li}",
                                        tag=f"h{lvl}_{li}")
                        sc = tokS[:O, 0:1] if (
                            tokS is not None and act == AF.Tanh) else 1.0
                        nc.scalar.activation(ho[:O, :bw], mp[:O, :bw], act,
                                             bias=bcol[:, :], scale=sc)
                        cur = [(ho, O)]
                        cc0 = 0
                    elif dst == "stream_out":
                        ob = tpool.tile([O, blk], f32,
                                        name=f"ob{g}{b0}", tag="ob")
                        nc.scalar.activation(ob[:O, :bw], mp[:O, :bw], act,
                                             bias=bcol[:, :])
                        base = stream_g * N0G + b0
                        for i in range(O):
                            nc.sync.dma_start(
                                P["out"].ap()[base : base + bw, i],
                                ob[i : i + 1, :bw])
                    else:
                        nc.scalar.activation(dst[:O, b0 : b0 + bw],
                                             mp[:O, :bw], act, bias=bcol[:, :])

        def alloc_hT(g, lvl, rows_list, nt):
            return [(gpool.tile([r, nt], bf16, name=f"hT{lvl}{g}_{i}",
                                tag=f"hT{lvl}_{i}"), r)
                    for i, r in enumerate(rows_list)]

        # natural+ones xe chunks from feature-major h via xbar transposes
        def to_nat_aug(g, lvl, hT, O, nsrc):
            outs = []
            for j in range(nsrc // 128):
                t = gpool.tile([128, O + 1], bf16, name=f"xn{lvl}{g}{j}",
                               tag=f"xn{lvl}_{j}")
                nc.sync.dma_start_transpose(
                    t[:, :O], hT[:O, j * 128 : (j + 1) * 128])
                nc.vector.memset(t[:, O : O + 1], 1.0)
                outs.append((t, 128))
            return outs

        _tok = [0]

        def make_token(samples, value):
            """ACT op chain: zeros/ones [P,8] tile that depends on reading
            one sample AP per source. samples: list of APs with equal
            partition count P."""
            _tok[0] += 1
            t = None
            for i, ap in enumerate(samples):
                nt_ = npool.tile([ap.shape[0], 8], f32,
                                 name=f"tok{_tok[0]}_{i}", tag=f"tok{i}")
                nc.scalar.activation(
                    nt_[:, : ap.shape[1]], ap, AF.Identity, scale=0.0,
                    bias=value if t is None else t[:, 0:1])
                t = nt_
            return t

        def hsamp(tile, ncols, nblk, P=None):
            v = tile[: (P or tile.shape[0]), :].rearrange(
                "p (a b) -> p a b", a=nblk)
            return v[:, :, 0]

        st = [dict() for _ in range(GRAPHS_PER_CORE)]

        # ===== level 3 =====
        for g in range(GRAPHS_PER_CORE):
            xe3 = gpool.tile([64, 257], bf16, name=f"xe3{g}", tag="xe3")
            nc.gpsimd.dma_start(xe3[:, :256],
                                P["x"].ap()[g * 64 : g * 64 + 64, :])
            nc.vector.memset(xe3[:, 256:257], 1.0)
            pT3 = build_pos18(P["pos"], g * N3G, N3G, False, f"p3{g}", use_dve=True)
            hT3 = alloc_hT(g, 3, [128, 128, 128], N2G)
            interp_level(g, 3, 64, N2G, 4, 256, 128, [(xe3, 64)], P["ps2"],
                         pT3, P["xs2"], hT3, n_dve=(0, 2))
            st[g]["hT3"] = hT3

        W3aT, b3a = prep_linear("W3a", "b3a", 128, 384, [128, 128, 128])
        W3bT, b3b = prep_linear("W3b", "b3b", 128, 128, [128])
        W2aT, b2a = prep_linear("W2a", "b2a", 64, 192, [128, 64])
        W2bT, b2b = prep_linear("W2b", "b2b", 64, 64, [64])
        W1aT, b1a = prep_linear("W1a", "b1a", 64, 67, [67])
        W1bT, b1b = prep_linear("W1b", "b1b", 64, 64, [64])
        W1cT, b1c = prep_linear("W1c", "b1c", 3, 64, [64])
        for g in range(GRAPHS_PER_CORE):
            tokS3 = make_token([hsamp(st[g]["hT3"][2][0], 2, 2)], 1.0)
            h3T = gpool.tile([128, N2G], bf16, name=f"h3T{g}", tag="h3T")
            mlp_phase(g, 3, st[g]["hT3"], N2G,
                      [(W3aT, b3a, 128, AF.Tanh, None),
                       (W3bT, b3b, 128, AF.Identity, h3T)], tokS=tokS3)
            st[g]["h3T"] = h3T
            st[g]["h3nat"] = to_nat_aug(g, 3, h3T, 128, N2G)

        # ===== level 2 =====
        for g in range(GRAPHS_PER_CORE):
            pT2 = build_pos18(P["ps2"], g * N2G, N2G, False, f"p2{g}", use_dve=True)
            hT2 = alloc_hT(g, 2, [128, 64], N1G)
            interp_level(g, 2, N2G, N1G, 8, 128, 64, st[g]["h3nat"], P["ps1"],
                         pT2, P["xs1"], hT2, n_dve=(8, 0))
            st[g]["hT2"] = hT2
        for g in range(GRAPHS_PER_CORE):
            tokS2 = make_token(
                [st[g]["hT2"][0][0][:64, 6 * 128 - 1 : 6 * 128]], 1.0)
            h2T = gpool.tile([64, N1G], bf16, name=f"h2T{g}", tag="h2T")
            mlp_phase(g, 2, st[g]["hT2"], N1G,
                      [(W2aT, b2a, 64, AF.Tanh, None),
                       (W2bT, b2b, 64, AF.Identity, h2T)], tokS=tokS2)
            st[g]["h2nat"] = to_nat_aug(g, 2, h2T, 64, N1G)

        # ===== level 1 =====
        tokZ1 = make_token([hsamp(st[g]["h2nat"][j][0], 1, 1)
                            for g in range(GRAPHS_PER_CORE) for j in (3, 7)],
                           0.0)
        for g in range(GRAPHS_PER_CORE):
            pT1 = build_pos18(P["ps1"], g * N1G, N1G, False, f"p1{g}", use_dve=True)
            hT1 = alloc_hT(g, 1, [67], N0G)
            interp_level(g, 1, N1G, N0G, 16, 64, 3, st[g]["h2nat"], P["ps0"],
                         pT1, P["xs0"], hT1, tokZ=tokZ1, n_dve=(0, 6))
            st[g]["hT1"] = hT1
        for g in range(GRAPHS_PER_CORE):
            tokS1 = make_token(
                [st[g]["hT1"][0][0][:64, (32 - 6) * 128 - 1 : (32 - 6) * 128]],
                1.0)
            mlp_phase(g, 1, st[g]["hT1"], N0G,
                      [(W1aT, b1a, 64, AF.Tanh, None),
                       (W1bT, b1b, 64, AF.Tanh, None),
                       (W1cT, b1c, 3, AF.Identity, "stream_out")],
                      stream_g=g, tokS=tokS1)

    return nc, P


_NC = None


def _patch_act_tables(arch):
    """Make Ln and Exp both resolve to natural_log_exp_and_others (which
    genuinely contains both) instead of two disjoint tables, so the
    ln->exp->ln sequence does not reload the ACT function table per op.
    Only restricts the placement choice; every emitted set id still names a
    table that really contains the required function."""
    from concourse.hw_specs import get_activation_tables
    tabs = get_activation_tables(arch)
    both = tabs.get("natural_log_exp_and_others")
    if not both:
        return
    for nm, st in tabs.items():
        if nm == "natural_log_exp_and_others":
            continue
        ln = next((x for x in st if str(x).lower().endswith("ln")), None)
        ex = next((x for x in st if str(x).lower().endswith("exp")), None)
        if ln in both:
            st.discard(ln)
        if ex in both:
            st.discard(ex)


def _get_nc():
    global _NC
    if _NC is None:
        nc = build_module()[0]
        _patch_act_tables(nc.m.arch)
        nc.finalize()
        _NC = nc
    return _NC


def shard_inputs(inputs):
    f = lambda name: np.ascontiguousarray(np.asarray(inputs[name], np.float32))
    arrs = {
        "x": (f("x"), N3G), "pos": (f("pos"), N3G),
        "xs2": (f("x_skip2"), N2G), "ps2": (f("pos_skip2"), N2G),
        "xs1": (f("x_skip1"), N1G), "ps1": (f("pos_skip1"), N1G),
        "xs0": (f("x_skip0"), N0G), "ps0": (f("pos_skip0"), N0G),
    }
    weights = {k: f(k) for k in ["W3a", "b3a", "W3b", "b3b", "W2a", "b2a",
                                 "W2b", "b2b", "W1a", "b1a", "W1b", "b1b",
                                 "W1c", "b1c"]}
    in_maps = []
    for c in range(N_CORES):
        m = dict(weights)
        for nm, (arr, ng) in arrs.items():
            m[nm] = np.ascontiguousarray(
                arr[2 * c * ng : (2 * c + 2) * ng])
        in_maps.append(m)
    return in_maps


def kernel(**inputs):
    nc = _get_nc()
    in_maps = shard_inputs(inputs)
    from concourse.bass_utils import run_bass_kernel_spmd

    res = run_bass_kernel_spmd(nc, in_maps, list(range(N_CORES)))
    return np.concatenate([np.asarray(r["out"], np.float32)
                           for r in res.results], axis=0)


if __name__ == "__main__":
    nc, _ = build_module()
    print("build ok")


# revision 41
# speedup vs baseline: 1.7865x; 1.0725x over previous
"""Trainium2 Bass kernel for nn_DecoderPp (PointNet++-style 3-level KNN decoder).

Data-parallel over 16 graphs: core g owns graphs 2g, 2g+1. Per level:
- PE computes s = |q-p|^2 + eps via one K=19 bf16 matmul pair per 512-col
  block: positions and squared norms are split hi/lo (q = qh + ql in bf16;
  norms 3-way) so every bf16 product is exact and |s - d^2| ~ 1e-5, at 1
  cycle/row instead of f32's 4.
- Dense weights wpos = 1/d^2 come either from ACT as exp(-ln(s)) or from DVE
  reciprocal, chosen per chunk: ln lives in a different ACT table than tanh,
  so chunks that must overlap a tanh (MLP) phase use the DVE path. The table
  catalog is patched so Ln and Exp both resolve to natural_log_exp_and_others
  (which genuinely holds both), and phase order is enforced with token tiles:
  Ln ops read a zeros-token (bias) produced from the previous MLP's outputs,
  Tanh ops read a ones-token (scale) produced from this level's hT samples.
- Selection runs on wpos (positive, so nearest = largest and 0 is a safe zap
  value): k=4/8 use one max8; k=16 uses 6 block max8s, then exact
  top8-of-48 + match_replace(0) + max8 for ranks 9-16 (rarely a block hides
  >8 of the true top-16; measured effect ~5e-4 rel err).
- One DVE scalar_tensor_tensor masks: W = (wpos >= wtau) * wpos, bf16.
- Aggregation is target-major: y[128, C+1] = sum_j WT_j^T @ xe_aug_j where
  xe_aug carries a trailing ones column so y[:, C] = sum(w) for free; W^T
  chunks come from xbar DMA transposes.
- y rows are normalized by 1/sumw into haug bf16 (ACT on L1, DVE elsewhere);
  skip features are DMA-copied from a per-level bf16 precast; PE transposes
  build the feature-major MLP input hT, drained from PSUM by ACT copies.
- MLPs run feature-major over 512-column blocks with tanh/bias fused on ACT;
  the last level streams its output per block straight to DRAM.
Built on Bacc (finalize() legalizes multi-semaphore waits via EVSEM).
"""
import sys
from contextlib import ExitStack

if "/opt/trn_rl_repo" not in sys.path:
    sys.path.insert(0, "/opt/trn_rl_repo")

import numpy as np

import concourse.bass as bass
import concourse.mybir as mybir
from concourse.bacc import Bacc
from concourse.tile import TileContext
from concourse.masks import make_identity

dt = mybir.dt
AF = mybir.ActivationFunctionType
ALU = mybir.AluOpType

N_CORES = 8
GRAPHS_PER_CORE = 2
N3G, N2G, N1G, N0G = 64, 256, 1024, 4096  # per-graph sizes per level

f32 = dt.float32
bf16 = dt.bfloat16


def build_module(exact_sel=False):
    nc = Bacc()

    P = {}

    def param(name, shape, out=False):
        P[name] = nc.declare_dram_parameter(name, list(shape), f32, isOutput=out)

    param("x", (GRAPHS_PER_CORE * N3G, 256))
    param("pos", (GRAPHS_PER_CORE * N3G, 3))
    param("xs2", (GRAPHS_PER_CORE * N2G, 128))
    param("ps2", (GRAPHS_PER_CORE * N2G, 3))
    param("xs1", (GRAPHS_PER_CORE * N1G, 64))
    param("ps1", (GRAPHS_PER_CORE * N1G, 3))
    param("xs0", (GRAPHS_PER_CORE * N0G, 3))
    param("ps0", (GRAPHS_PER_CORE * N0G, 3))
    for nm, shp in [
        ("W3a", (128, 384)), ("b3a", (128,)),
        ("W3b", (128, 128)), ("b3b", (128,)),
        ("W2a", (64, 192)), ("b2a", (64,)),
        ("W2b", (64, 64)), ("b2b", (64,)),
        ("W1a", (64, 67)), ("b1a", (64,)),
        ("W1b", (64, 64)), ("b1b", (64,)),
        ("W1c", (3, 64)), ("b1c", (3,)),
    ]:
        param(nm, shp)
    param("out", (GRAPHS_PER_CORE * N0G, 3), out=True)

    with TileContext(nc) as tc, ExitStack() as ctx:
        consts = ctx.enter_context(tc.tile_pool(name="consts", bufs=1))
        wpool = ctx.enter_context(tc.tile_pool(name="weights", bufs=1))
        gpool = ctx.enter_context(tc.tile_pool(name="graph", bufs=2))
        tpool = ctx.enter_context(tc.tile_pool(name="tiles", bufs=3))
        npool = ctx.enter_context(tc.tile_pool(name="narrow", bufs=8))
        psS = ctx.enter_context(tc.tile_pool(name="ps_s", bufs=2, space="PSUM"))
        psY = ctx.enter_context(tc.tile_pool(name="ps_y", bufs=1, space="PSUM"))
        psT = ctx.enter_context(tc.tile_pool(name="ps_tp", bufs=1, space="PSUM"))
        psM = ctx.enter_context(tc.tile_pool(name="ps_mlp", bufs=1, space="PSUM"))

        ident = consts.tile([128, 128], f32)
        make_identity(nc, ident)
        identb = consts.tile([128, 128], bf16)
        nc.scalar.activation(identb[:, :], ident[:, :], AF.Copy)

        # ---- weight prep: bf16 natural via casting DMA, then xbar
        # transposes into [cw, O] lhsT chunks; f32 bias columns. ----
        def prep_linear(wname, bname, O, I, splits):
            # xbar transpose needs partitions %16 == 0 and free %128 == 0
            Op = (O + 15) // 16 * 16
            Ip = (I + 127) // 128 * 128
            wbf = wpool.tile([Op, Ip], bf16, name=f"{wname}bf",
                             tag=f"{wname}bf")
            if Op != O or Ip != I:
                nc.vector.memset(wbf[:, :], 0.0)
            nc.gpsimd.dma_start(wbf[:O, :I], P[wname].ap())
            chunks = []
            c0 = 0
            for j, cw in enumerate(splits):
                wt = wpool.tile([128, Op], bf16, name=f"{wname}T{j}",
                                tag=f"{wname}T{j}")
                nc.sync.dma_start_transpose(
                    wt[:, :], wbf[:, j * 128 : (j + 1) * 128])
                chunks.append((wt, cw))
                c0 += cw
            bcol = wpool.tile([O, 1], f32, name=f"{bname}c", tag=f"{bname}c")
            nc.sync.dma_start(bcol[:, :], P[bname].ap())
            return chunks, bcol


        # ---- [5, n] position-feature rows from natural [n, 3] ----
        # q-side rows: [2q; -|q|^2; 1], p-side rows: [p; 1; -|p|^2]
        _p5n = [0]
        KD = 19  # hi/lo bf16 rows: 4x3 cross, 3+3 split norms, 1 eps floor

        def build_pos18(dram, base, n, qside, tag, use_dve=False):
            """[18, n] bf16 position rows from natural [n, 3] f32.
            Pairing (q-row, p-row): (-2qh,ph)x3 (-2ql,ph)x3 (-2qh,pl)x3
            (-2ql,pl)x3 (q2a,1)(q2b,1)(q2c,1) (1,p2a)(1,p2b)(1,p2c), so
            q18^T . p18 = |q-p|^2 with ~1e-5 abs error (products of bf16
            pairs are exact; only the residual roundings remain)."""
            _p5n[0] += 1
            tag = tag + ("q" if qside else "p")
            uid = f"{tag}_{_p5n[0]}"
            rows = min(n, 128)
            a = max(1, n // 128)
            pn = gpool.tile([rows, a * 3], f32, name=f"pn{uid}", tag=f"pn{tag}")
            if n >= 128:
                nc.sync.dma_start(
                    pn[:, :],
                    dram.ap()[base : base + n, :].rearrange(
                        "(a p) d -> p a d", p=128))
            else:
                nc.sync.dma_start(pn[:, :], dram.ap()[base : base + n, :])
            ph = gpool.tile([rows, a * 3], bf16, name=f"ph{uid}", tag=f"ph{tag}")
            if use_dve:
                nc.vector.tensor_copy(ph[:, :], pn[:, :])
            else:
                nc.scalar.activation(ph[:, :], pn[:, :], AF.Copy)
            pl = gpool.tile([rows, a * 3], bf16, name=f"pl{uid}", tag=f"pl{tag}")
            nc.vector.tensor_tensor(pl[:, :], pn[:, :], ph[:, :],
                                    op=ALU.subtract)
            sq = gpool.tile([rows, a * 3], f32, name=f"sq{uid}", tag=f"sq{tag}")
            if use_dve:
                nc.vector.tensor_tensor(sq[:, :], pn[:, :], pn[:, :],
                                        op=ALU.mult)
            else:
                nc.scalar.activation(sq[:, :], pn[:, :], AF.Square)
            sq3 = sq[:, :].rearrange("p (a d) -> p a d", d=3)
            qq = gpool.tile([rows, a], f32, name=f"qq{uid}", tag=f"qq{tag}")
            nc.vector.tensor_tensor(qq[:, :], sq3[:, :, 0], sq3[:, :, 1],
                                    op=ALU.add)
            nc.vector.tensor_tensor(qq[:, :], qq[:, :], sq3[:, :, 2],
                                    op=ALU.add)
            # 3-way split of the squared norm
            n2a = gpool.tile([rows, a], bf16, name=f"n2a{uid}", tag=f"n2a{tag}")
            if use_dve:
                nc.vector.tensor_copy(n2a[:, :], qq[:, :])
            else:
                nc.scalar.activation(n2a[:, :], qq[:, :], AF.Copy)
            r1 = gpool.tile([rows, a], f32, name=f"r1{uid}", tag=f"r1{tag}")
            nc.vector.tensor_tensor(r1[:, :], qq[:, :], n2a[:, :],
                                    op=ALU.subtract)
            n2b = gpool.tile([rows, a], bf16, name=f"n2b{uid}", tag=f"n2b{tag}")
            if use_dve:
                nc.vector.tensor_copy(n2b[:, :], r1[:, :])
            else:
                nc.scalar.activation(n2b[:, :], r1[:, :], AF.Copy)
            n2c = gpool.tile([rows, a], bf16, name=f"n2c{uid}", tag=f"n2c{tag}")
            nc.vector.tensor_tensor(n2c[:, :], r1[:, :], n2b[:, :],
                                    op=ALU.subtract)
            p5 = gpool.tile([rows, a * KD], bf16, name=f"p5{uid}",
                            tag=f"p5{tag}")
            p5v = p5[:, :].rearrange("p (a d) -> p a d", d=KD)
            hsc = -2.0 if qside else 1.0
            ph3 = ph[:, :].rearrange("p (a d) -> p a d", d=3)
            pl3 = pl[:, :].rearrange("p (a d) -> p a d", d=3)

            def scat(dst, src, scale):
                if use_dve:
                    if scale == 1.0:
                        nc.vector.tensor_copy(dst, src)
                    else:
                        nc.vector.tensor_scalar(dst, src, scale, None,
                                                op0=ALU.mult)
                else:
                    nc.scalar.activation(dst, src, AF.Copy, scale=scale)

            if qside:
                scat(p5v[:, :, 0:3], ph3[:, :, :], hsc)
                scat(p5v[:, :, 3:6], pl3[:, :, :], hsc)
                scat(p5v[:, :, 6:9], ph3[:, :, :], hsc)
                scat(p5v[:, :, 9:12], pl3[:, :, :], hsc)
            else:
                scat(p5v[:, :, 0:3], ph3[:, :, :], 1.0)
                scat(p5v[:, :, 3:6], ph3[:, :, :], 1.0)
                scat(p5v[:, :, 6:9], pl3[:, :, :], 1.0)
                scat(p5v[:, :, 9:12], pl3[:, :, :], 1.0)
            sq0, one0 = (12, 15) if qside else (15, 12)
            for i, t in enumerate((n2a, n2b, n2c)):
                scat(p5v[:, :, sq0 + i : sq0 + i + 1],
                     t[:, :].rearrange("p (a d) -> p a d", d=1), 1.0)
            nc.vector.memset(p5v[:, :, one0 : one0 + 3], 1.0)
            # row 18: (eps, 1) pair floors s at +eps so ln never sees <= 0
            nc.vector.memset(p5v[:, :, 18:19], 1.0)
            if qside:
                scat(p5v[:, :, 18:19], p5v[:, :, 18:19], 2e-4)
            pT = gpool.tile([KD, n], bf16, name=f"pT{uid}", tag=f"pT{tag}")
            c = 0
            while c < a:
                w = min(2, a - c)
                tp = psT.tile([128, 256], bf16, name=f"tp5{uid}{c}", tag="tp5")
                for j in range(w):
                    nc.tensor.transpose(
                        tp[:KD, j * 128 : j * 128 + rows],
                        p5[:, (c + j) * KD : (c + j) * KD + KD],
                        identb[:rows, :rows])
                wd = 256 if w == 2 else rows
                nc.scalar.activation(pT[:, c * 128 : c * 128 + wd],
                                     tp[:KD, :wd], AF.Copy)
                c += w
            return pT

        # ---------------- interpolation level ----------------
        def interp_level(g, lvl, ns, nt, k, Cs, Ck, xe_chunks, q_dram, pT,
                         skip_dram, hT_chunks, tokZ=None, n_dve=(0, 0)):
            n_sch = max(1, ns // 128)
            Ctot = Cs + Ck
            fsplits = []
            f0 = 0
            while f0 < Ctot:
                fw = min(128, Ctot - f0)
                fsplits.append((f0, fw))
                f0 += fw
            pend = {}
            sub = min(nt, 1024)
            skipbf = gpool.tile([128, (nt // 128) * Ck], bf16,
                                name=f"skb{lvl}{g}", tag=f"skb{lvl}")
            nc.gpsimd.dma_start(
                skipbf[:, :],
                skip_dram.ap()[g * nt : (g + 1) * nt, :].rearrange(
                    "(a p) d -> p a d", p=128))
            q5T_next = build_pos18(q_dram, g * nt, sub, True, f"q{lvl}",
                                   use_dve=True)
            for r0 in range(0, nt, sub):
                q5T = q5T_next
                if r0 + sub < nt:
                    q5T_next = build_pos18(q_dram, g * nt + r0 + sub, sub,
                                           True, f"q{lvl}",
                                           use_dve=(lvl != 1 or g == 0 and
                                                    r0 == 0))
                interp_sub(g, lvl, ns, nt, k, Cs, Ck, Ctot, xe_chunks, q5T,
                           pT, skip_dram, hT_chunks, fsplits, pend, r0, sub,
                           n_sch, skipbf, tokZ, n_dve)

        def interp_sub(g, lvl, ns, nt, k, Cs, Ck, Ctot, xe_chunks, q5T, pT,
                       skip_dram, hT_chunks, fsplits, pend, r0, sub, n_sch,
                       skipbf, tokZ, n_dve):
            for ti in range(r0 // 128, (r0 + sub) // 128):
                t0 = ti * 128
                s = psS.tile([128, 1024], f32, name=f"s{lvl}{g}{ti}", tag="s")
                for h0 in range(0, ns, 512):
                    h1 = min(ns, h0 + 512)
                    nc.tensor.matmul(s[:, h0:h1],
                                     q5T[:, t0 - r0 : t0 - r0 + 128],
                                     pT[:, h0:h1], start=True, stop=True)

                wpos = tpool.tile([128, ns], f32, name=f"wp{lvl}{g}{ti}",
                                  tag=f"wp{lvl}", bufs=2)
                nchunks = nt // 128
                head, tail = n_dve
                if ti < head or ti >= nchunks - tail:
                    # DVE reciprocal path: no table-bound ACT ops, so these
                    # chunks overlap freely with the previous tanh phase
                    nc.vector.reciprocal(wpos[:, :], s[:, :ns])
                else:
                    L = tpool.tile([128, ns], f32, name=f"L{lvl}{g}{ti}",
                                   tag=f"L{lvl}", bufs=2)
                    nc.scalar.activation(
                        L[:, :], s[:, :ns], AF.Ln,
                        bias=tokZ[:, 0:1] if tokZ is not None else 0.0)
                    nc.scalar.activation(wpos[:, :], L[:, :], AF.Exp,
                                         scale=-1.0)

                if k == 16 and not exact_sel:
                    NB = 6
                    bnd = [round(ns * i / NB) for i in range(NB + 1)]
                    cand = tpool.tile([128, 8 * NB], f32, name=f"cd{g}{ti}",
                                      tag="cand")
                    for b in range(NB):
                        nc.vector.max(cand[:, 8 * b : 8 * b + 8],
                                      wpos[:, bnd[b] : bnd[b + 1]])
                    v8 = npool.tile([128, 8], f32, name=f"v8{g}{ti}", tag="v8")
                    nc.vector.max(v8[:, :], cand[:, :])
                    zap = tpool.tile([128, 8 * NB], f32, name=f"zp{g}{ti}",
                                     tag="zap")
                    nc.vector.match_replace(zap[:, :], v8[:, :], cand[:, :],
                                            0.0)
                    v16 = npool.tile([128, 8], f32, name=f"v16{g}{ti}",
                                     tag="v16")
                    nc.vector.max(v16[:, :], zap[:, :])
                    wtau = v16[:, 7:8]
                elif k == 16:
                    v8 = npool.tile([128, 8], f32, name=f"v8{g}{ti}", tag="v8")
                    nc.vector.max(v8[:, :], wpos[:, :])
                    zap = tpool.tile([128, ns], f32, name=f"zp{g}{ti}",
                                     tag="zap")
                    nc.vector.match_replace(zap[:, :], v8[:, :], wpos[:, :],
                                            0.0)
                    v16 = npool.tile([128, 8], f32, name=f"v16{g}{ti}",
                                     tag="v16")
                    nc.vector.max(v16[:, :], zap[:, :])
                    wtau = v16[:, 7:8]
                else:
                    v8 = npool.tile([128, 8], f32, name=f"v8{g}{ti}", tag="v8")
                    nc.vector.max(v8[:, :], wpos[:, :])
                    wtau = v8[:, k - 1 : k]

                ns_pad = max(128, ns)
                W = tpool.tile([128, ns_pad], bf16, name=f"W{lvl}{g}{ti}",
                               tag=f"W{lvl}")
                nc.vector.scalar_tensor_tensor(W[:, :ns], wpos[:, :], wtau,
                                               wpos[:, :], op0=ALU.is_ge,
                                               op1=ALU.mult)
                if ns < ns_pad:
                    nc.vector.memset(W[:, ns:], 0.0)
                WT = []
                for j in range(n_sch):
                    kr = min(128, ns)
                    wt = tpool.tile([128, 128], bf16, name=f"WT{j}g{g}{ti}",
                                    tag=f"WT{lvl}_{j}")
                    nc.sync.dma_start_transpose(
                        wt[:, :], W[:, j * 128 : (j + 1) * 128])
                    WT.append((wt, kr))

                y = psY.tile([128, 257], f32, name=f"y{lvl}{g}{ti}", tag="y")
                for j, (wt, kr) in enumerate(WT):
                    nc.tensor.matmul(
                        y[:, : Cs + 1], wt[:kr, :],
                        xe_chunks[j][0][:kr, : Cs + 1],
                        start=(j == 0), stop=(j == n_sch - 1))
                sw = npool.tile([128, 1], f32, name=f"sw{g}{ti}", tag="sw")
                nc.vector.reciprocal(sw[:, :], y[:, Cs : Cs + 1])
                haug = tpool.tile([128, Ctot], bf16, name=f"ha{lvl}{g}{ti}",
                                  tag=f"ha{lvl}")
                if lvl == 1:
                    nc.scalar.activation(haug[:, :Cs], y[:, :Cs], AF.Copy,
                                         scale=sw[:, :])
                else:
                    nc.vector.tensor_scalar(haug[:, :Cs], y[:, :Cs],
                                            sw[:, :], None, op0=ALU.mult)
                nc.sync.dma_start(haug[:, Cs:Ctot],
                                  skipbf[:, ti * Ck : (ti + 1) * Ck])

                # transpose haug -> hT; batch 2 chunks per drain when the
                # psum period tile (1 bank) has room
                period = max(1, 512 // (128 * len(fsplits)))
                pw = 128 * period
                if not pend.get("t0s"):
                    pend["tile"] = psT.tile([128, pw * len(fsplits)], bf16,
                                            name=f"tph{lvl}{g}{ti}",
                                            tag="tph")
                    pend["t0s"] = []
                tp = pend["tile"]
                npos = len(pend["t0s"])
                for fi, (f0, fw) in enumerate(fsplits):
                    nc.tensor.transpose(
                        tp[:fw, fi * pw + npos * 128 :
                           fi * pw + npos * 128 + 128],
                        haug[:, f0 : f0 + fw], identb[:, :])
                pend["t0s"].append(t0)
                if len(pend["t0s"]) == period or ti == nt // 128 - 1:
                    nb = len(pend["t0s"])
                    for fi, (f0, fw) in enumerate(fsplits):
                        ht = hT_chunks[fi][0]
                        nc.scalar.activation(
                            ht[:, pend["t0s"][0] : pend["t0s"][0] + nb * 128],
                            tp[:fw, fi * pw : fi * pw + nb * 128], AF.Copy)
                    pend["t0s"] = []

        # ---------------- MLP over feature-major hT ----------------
        def mlp_phase(g, lvl, hT_chunks, nt, layers, blk=512, stream_g=0,
                      tokS=None):
            """layers: (chunks, bcol, O, act, dst). dst None = per-block
            scratch (consumed by the next layer); dst tile = full-width."""
            for b0 in range(0, nt, blk):
                bw = min(blk, nt - b0)
                cur = hT_chunks
                cc0 = b0
                for li, (chunks, bcol, O, act, dst) in enumerate(layers):
                    mp = psM.tile([128, 512], f32, name=f"m{lvl}{g}{b0}{li}",
                                  tag="mlp")
                    nkc = len(cur)
                    for j, (ct, kr) in enumerate(cur):
                        wt, cw = chunks[j]
                        nc.tensor.matmul(mp[:O, :bw], wt[:kr, :O],
                                         ct[:kr, cc0 : cc0 + bw],
                                         start=(j == 0), stop=(j == nkc - 1))
                    if dst is None:
                        ho = tpool.tile([O, blk], bf16,
                                        name=f"h{lvl}{g}{b0}_{li}",
                                        tag=f"h{lvl}_{li}")
                        sc = tokS[:O, 0:1] if (
                            tokS is not None and act == AF.Tanh) else 1.0
                        nc.scalar.activation(ho[:O, :bw], mp[:O, :bw], act,
                                             bias=bcol[:, :], scale=sc)
                        cur = [(ho, O)]
                        cc0 = 0
                    elif dst == "stream_out":
                        ob = tpool.tile([O, blk], f32,
                                        name=f"ob{g}{b0}", tag="ob")
                        nc.scalar.activation(ob[:O, :bw], mp[:O, :bw], act,
                                             bias=bcol[:, :])
                        base = stream_g * N0G + b0
                        for i in range(O):
                            nc.sync.dma_start(
                                P["out"].ap()[base : base + bw, i],
                                ob[i : i + 1, :bw])
                    else:
                        nc.scalar.activation(dst[:O, b0 : b0 + bw],
                                             mp[:O, :bw], act, bias=bcol[:, :])

        def alloc_hT(g, lvl, rows_list, nt):
            return [(gpool.tile([r, nt], bf16, name=f"hT{lvl}{g}_{i}",
                                tag=f"hT{lvl}_{i}"), r)
                    for i, r in enumerate(rows_list)]

        # natural+ones xe chunks from feature-major h via xbar transposes
        def to_nat_aug(g, lvl, hT, O, nsrc):
            outs = []
            for j in range(nsrc // 128):
                t = gpool.tile([128, O + 1], bf16, name=f"xn{lvl}{g}{j}",
                               tag=f"xn{lvl}_{j}")
                nc.sync.dma_start_transpose(
                    t[:, :O], hT[:O, j * 128 : (j + 1) * 128])
                nc.vector.memset(t[:, O : O + 1], 1.0)
                outs.append((t, 128))
            return outs

        _tok = [0]

        def make_token(samples, value):
            """ACT op chain: zeros/ones [P,8] tile that depends on reading
            one sample AP per source. samples: list of APs with equal
            partition count P."""
            _tok[0] += 1
            t = None
            for i, ap in enumerate(samples):
                nt_ = npool.tile([ap.shape[0], 8], f32,
                                 name=f"tok{_tok[0]}_{i}", tag=f"tok{i}")
                nc.scalar.activation(
                    nt_[:, : ap.shape[1]], ap, AF.Identity, scale=0.0,
                    bias=value if t is None else t[:, 0:1])
                t = nt_
            return t

        def hsamp(tile, ncols, nblk, P=None):
            v = tile[: (P or tile.shape[0]), :].rearrange(
                "p (a b) -> p a b", a=nblk)
            return v[:, :, 0]

        st = [dict() for _ in range(GRAPHS_PER_CORE)]

        # ===== level 3 =====
        for g in range(GRAPHS_PER_CORE):
            xe3 = gpool.tile([64, 257], bf16, name=f"xe3{g}", tag="xe3")
            nc.gpsimd.dma_start(xe3[:, :256],
                                P["x"].ap()[g * 64 : g * 64 + 64, :])
            nc.vector.memset(xe3[:, 256:257], 1.0)
            pT3 = build_pos18(P["pos"], g * N3G, N3G, False, f"p3{g}", use_dve=True)
            hT3 = alloc_hT(g, 3, [128, 128, 128], N2G)
            interp_level(g, 3, 64, N2G, 4, 256, 128, [(xe3, 64)], P["ps2"],
                         pT3, P["xs2"], hT3, n_dve=(0, 2))
            st[g]["hT3"] = hT3

        W3aT, b3a = prep_linear("W3a", "b3a", 128, 384, [128, 128, 128])
        W3bT, b3b = prep_linear("W3b", "b3b", 128, 128, [128])
        W2aT, b2a = prep_linear("W2a", "b2a", 64, 192, [128, 64])
        W2bT, b2b = prep_linear("W2b", "b2b", 64, 64, [64])
        W1aT, b1a = prep_linear("W1a", "b1a", 64, 67, [67])
        W1bT, b1b = prep_linear("W1b", "b1b", 64, 64, [64])
        W1cT, b1c = prep_linear("W1c", "b1c", 3, 64, [64])
        for g in range(GRAPHS_PER_CORE):
            tokS3 = make_token([hsamp(st[g]["hT3"][2][0], 2, 2)], 1.0)
            h3T = gpool.tile([128, N2G], bf16, name=f"h3T{g}", tag="h3T")
            mlp_phase(g, 3, st[g]["hT3"], N2G,
                      [(W3aT, b3a, 128, AF.Tanh, None),
                       (W3bT, b3b, 128, AF.Identity, h3T)], tokS=tokS3)
            st[g]["h3T"] = h3T
            st[g]["h3nat"] = to_nat_aug(g, 3, h3T, 128, N2G)

        # ===== level 2 =====
        for g in range(GRAPHS_PER_CORE):
            pT2 = build_pos18(P["ps2"], g * N2G, N2G, False, f"p2{g}", use_dve=True)
            hT2 = alloc_hT(g, 2, [128, 64], N1G)
            interp_level(g, 2, N2G, N1G, 8, 128, 64, st[g]["h3nat"], P["ps1"],
                         pT2, P["xs1"], hT2, n_dve=(8, 0))
            st[g]["hT2"] = hT2
        for g in range(GRAPHS_PER_CORE):
            tokS2 = make_token(
                [st[g]["hT2"][0][0][:64, 6 * 128 - 1 : 6 * 128]], 1.0)
            h2T = gpool.tile([64, N1G], bf16, name=f"h2T{g}", tag="h2T")
            mlp_phase(g, 2, st[g]["hT2"], N1G,
                      [(W2aT, b2a, 64, AF.Tanh, None),
                       (W2bT, b2b, 64, AF.Identity, h2T)], tokS=tokS2)
            st[g]["h2nat"] = to_nat_aug(g, 2, h2T, 64, N1G)

        # ===== level 1 =====
        tokZ1 = make_token([hsamp(st[g]["h2nat"][j][0], 1, 1)
                            for g in range(GRAPHS_PER_CORE) for j in (3, 7)],
                           0.0)
        for g in range(GRAPHS_PER_CORE):
            pT1 = build_pos18(P["ps1"], g * N1G, N1G, False, f"p1{g}", use_dve=True)
            hT1 = alloc_hT(g, 1, [67], N0G)
            interp_level(g, 1, N1G, N0G, 16, 64, 3, st[g]["h2nat"], P["ps0"],
                         pT1, P["xs0"], hT1, tokZ=tokZ1, n_dve=(2, 8))
            st[g]["hT1"] = hT1
        for g in range(GRAPHS_PER_CORE):
            tokS1 = make_token(
                [st[g]["hT1"][0][0][:64, (32 - 8) * 128 - 1 : (32 - 8) * 128]],
                1.0)
            mlp_phase(g, 1, st[g]["hT1"], N0G,
                      [(W1aT, b1a, 64, AF.Tanh, None),
                       (W1bT, b1b, 64, AF.Tanh, None),
                       (W1cT, b1c, 3, AF.Identity, "stream_out")],
                      stream_g=g, tokS=tokS1)

    return nc, P


_NC = None


def _patch_act_tables(arch):
    """Make Ln and Exp both resolve to natural_log_exp_and_others (which
    genuinely contains both) instead of two disjoint tables, so the
    ln->exp->ln sequence does not reload the ACT function table per op.
    Only restricts the placement choice; every emitted set id still names a
    table that really contains the required function."""
    from concourse.hw_specs import get_activation_tables
    tabs = get_activation_tables(arch)
    both = tabs.get("natural_log_exp_and_others")
    if not both:
        return
    for nm, st in tabs.items():
        if nm == "natural_log_exp_and_others":
            continue
        ln = next((x for x in st if str(x).lower().endswith("ln")), None)
        ex = next((x for x in st if str(x).lower().endswith("exp")), None)
        if ln in both:
            st.discard(ln)
        if ex in both:
            st.discard(ex)


def _get_nc():
    global _NC
    if _NC is None:
        nc = build_module()[0]
        _patch_act_tables(nc.m.arch)
        nc.finalize()
        _NC = nc
    return _NC


def shard_inputs(inputs):
    f = lambda name: np.ascontiguousarray(np.asarray(inputs[name], np.float32))
    arrs = {
        "x": (f("x"), N3G), "pos": (f("pos"), N3G),
        "xs2": (f("x_skip2"), N2G), "ps2": (f("pos_skip2"), N2G),
        "xs1": (f("x_skip1"), N1G), "ps1": (f("pos_skip1"), N1G),
        "xs0": (f("x_skip0"), N0G), "ps0": (f("pos_skip0"), N0G),
    }
    weights = {k: f(k) for k in ["W3a", "b3a", "W3b", "b3b", "W2a", "b2a",
                                 "W2b", "b2b", "W1a", "b1a", "W1b", "b1b",
                                 "W1c", "b1c"]}
    in_maps = []
    for c in range(N_CORES):
        m = dict(weights)
        for nm, (arr, ng) in arrs.items():
            m[nm] = np.ascontiguousarray(
                arr[2 * c * ng : (2 * c + 2) * ng])
        in_maps.append(m)
    return in_maps


def kernel(**inputs):
    nc = _get_nc()
    in_maps = shard_inputs(inputs)
    from concourse.bass_utils import run_bass_kernel_spmd

    res = run_bass_kernel_spmd(nc, in_maps, list(range(N_CORES)))
    return np.concatenate([np.asarray(r["out"], np.float32)
                           for r in res.results], axis=0)


if __name__ == "__main__":
    nc, _ = build_module()
    print("build ok")
